# revision 4
# baseline (speedup 1.0000x reference)
"""Autoregressive GRU on 8 TRN2 NeuronCores.

Data-parallel: batch B=512 is split as 64 rows per core; the small GRU
weights are replicated and the T=128 sequential loop runs locally per core.

Key algebra (Keras GRU, reset_after=True, gate order [z, r, h]):
  step 0:  inp = 0, h = x  ->  gx = b[0], gh = x @ U + b[1]
  step t>=1: inp == h      ->  gx + gh uses (W + U) for the z and r gates
so per step we need ONE matmul against a host-prefused weight matrix:
  V  = [Wr+Ur | Uh | Wh | Wz+Uz]   (steps >= 1)   [D, 4D]
  V0 = [Ur   | Uh | 0  | Uz    ]   (step 0)       [D, 4D]
with PSUM bank layout [rpre | hh | xh | zpre] (each D wide), then
  r = sigmoid(rpre); hhat = tanh(xh + r*hh); z = sigmoid(zpre)
  h_new = hhat + z*(h - hhat)
The state is kept twice: h [64, D] (batch-major, bf16) for the update and
hT [128, 4*64] (D-major K-chunked, bf16) as the matmul's stationary operand;
hT is rebuilt each step with 4 PE transposes.
"""

import numpy as np
import ml_dtypes

B, D, T = 512, 512, 128
NCORES = 8
BLOC = B // NCORES  # 64
P = 128
KC = D // P  # 4 K-chunks
GW = 4 * D  # 2048 gate columns: [r | hh | xh | z]
NB = GW // 512  # 4 psum banks

_BF16 = ml_dtypes.bfloat16

# set by test harness to capture a profile; harmless when False
TRACE = False
TMPDIR = None
LAST = {}


def _prepare_weights(W, U, b):
    """Host-side fusion. Returns device-layout arrays (numpy)."""
    Wz, Wr, Wh = W[:, :D], W[:, D : 2 * D], W[:, 2 * D :]
    Uz, Ur, Uh = U[:, :D], U[:, D : 2 * D], U[:, 2 * D :]
    V = np.concatenate([Wr + Ur, Uh, Wh, Wz + Uz], axis=1)  # [D, GW]
    V0 = np.concatenate([Ur, Uh, np.zeros_like(Wh), Uz], axis=1)
    b0, b1 = b[0], b[1]
    bias = np.concatenate(
        [b0[D : 2 * D] + b1[D : 2 * D], b1[2 * D :], b0[2 * D :], b0[:D] + b1[:D]]
    )  # [GW], order [r | hh | xh | z]
    return V, V0, bias


def _dev_layout(V):
    # V_dev[p, k*GW + j] = V[k*128 + p, j]
    return np.ascontiguousarray(
        V.reshape(KC, P, GW).transpose(1, 0, 2).reshape(P, KC * GW)
    )


_CACHE = {}


def _build(has_bias: bool):
    import concourse.mybir as mybir
    import concourse.tile as tile
    from concourse import bacc
    from concourse.masks import make_identity

    f32 = mybir.dt.float32
    bf16 = mybir.dt.bfloat16
    AF = mybir.ActivationFunctionType

    nc = bacc.Bacc(
        "TRN2", target_bir_lowering=False, debug=False, num_devices=NCORES
    )
    v0_d = nc.dram_tensor("v0", [P, KC * GW], bf16, kind="ExternalInput").ap()
    v_d = nc.dram_tensor("v", [P, KC * GW], bf16, kind="ExternalInput").ap()
    h0_d = nc.dram_tensor("h0", [BLOC, D], bf16, kind="ExternalInput").ap()
    h0T_d = nc.dram_tensor("h0T", [P, KC * BLOC], bf16, kind="ExternalInput").ap()
    if has_bias:
        bias_d = nc.dram_tensor("bias", [BLOC, GW], f32, kind="ExternalInput").ap()
    out_d = nc.dram_tensor("out", [BLOC, T, D], f32, kind="ExternalOutput").ap()

    with tile.TileContext(nc) as tc:
        with (
            tc.tile_pool(name="const", bufs=1) as cpool,
            tc.tile_pool(name="state", bufs=2) as spool,
            tc.tile_pool(name="work", bufs=3) as wpool,
            tc.tile_pool(name="outp", bufs=3) as opool,
            tc.tile_pool(name="gates", bufs=1, space="PSUM") as gpool,
            tc.tile_pool(name="trp", bufs=2, space="PSUM") as trpool,
        ):
            v0_sb = cpool.tile([P, KC * GW], bf16, tag="v0")
            v_sb = cpool.tile([P, KC * GW], bf16, tag="v")
            ident = cpool.tile([BLOC, BLOC], bf16, tag="ident")
            nc.sync.dma_start(v0_sb[:], v0_d[:])
            make_identity(nc, ident[:])

            h = spool.tile([BLOC, D], bf16, tag="h")
            hT = spool.tile([P, KC * BLOC], bf16, tag="hT")
            nc.sync.dma_start(h[:], h0_d[:])
            nc.sync.dma_start(hT[:], h0T_d[:])
            nc.sync.dma_start(v_sb[:], v_d[:])
            if has_bias:
                bias_sb = cpool.tile([BLOC, GW], f32, tag="bias")
                nc.sync.dma_start(bias_sb[:], bias_d[:])

            for t in range(T):
                vsb = v0_sb if t == 0 else v_sb
                g = gpool.tile([BLOC, GW], f32, tag="g")  # banks [r|hh|xh|z]
                for n in range(NB):
                    for k in range(KC):
                        nc.tensor.matmul(
                            g[:, n * 512 : (n + 1) * 512],
                            hT[:, k * BLOC : (k + 1) * BLOC],
                            vsb[:, k * GW + n * 512 : k * GW + (n + 1) * 512],
                            start=(k == 0),
                            stop=(k == KC - 1),
                        )
                if has_bias:
                    nc.vector.tensor_add(g[:], g[:], bias_sb[:])

                r = wpool.tile([BLOC, D], bf16, tag="r")
                nc.scalar.activation(r[:], g[:, 0:512], AF.Sigmoid)
                p = wpool.tile([BLOC, D], bf16, tag="p")
                nc.vector.tensor_mul(p[:], r[:], g[:, 512:1024])
                q = wpool.tile([BLOC, D], bf16, tag="q")
                nc.vector.tensor_add(q[:], p[:], g[:, 1024:1536])
                hhat = wpool.tile([BLOC, D], bf16, tag="hhat")
                nc.scalar.activation(hhat[:], q[:], AF.Tanh)
                s = wpool.tile([BLOC, D], bf16, tag="s")
                nc.vector.tensor_sub(s[:], h[:], hhat[:])
                z = wpool.tile([BLOC, D], bf16, tag="z")
                nc.scalar.activation(z[:], g[:, 1536:2048], AF.Sigmoid)
                tt = wpool.tile([BLOC, D], bf16, tag="t")
                nc.vector.tensor_mul(tt[:], z[:], s[:])
                h_new = spool.tile([BLOC, D], bf16, tag="h")
                nc.vector.tensor_add(h_new[:], hhat[:], tt[:])

                of = opool.tile([BLOC, D], f32, tag="of")
                nc.scalar.copy(of[:], h_new[:])
                nc.sync.dma_start(out_d[:, t, :], of[:])

                trp = trpool.tile([P, KC * BLOC], bf16, tag="trp")
                for k in range(KC):
                    nc.tensor.transpose(
                        trp[:, k * BLOC : (k + 1) * BLOC],
                        h_new[:, k * P : (k + 1) * P],
                        ident[:],
                    )
                hT_new = spool.tile([P, KC * BLOC], bf16, tag="hT")
                nc.vector.tensor_copy(hT_new[:], trp[:])
                h, hT = h_new, hT_new

    nc.compile()
    return nc


def kernel(x, W, U, b):
    from concourse.bass_utils import run_bass_kernel_spmd

    x = np.asarray(x, dtype=np.float32)
    W = np.asarray(W, dtype=np.float32)
    U = np.asarray(U, dtype=np.float32)
    b = np.asarray(b, dtype=np.float32)

    V, V0, bias = _prepare_weights(W, U, b)
    has_bias = bool(np.any(bias != 0.0))
    v_dev = _dev_layout(V).astype(_BF16)
    v0_dev = _dev_layout(V0).astype(_BF16)

    key = ("gru", has_bias)
    if key not in _CACHE:
        _CACHE[key] = _build(has_bias)
    nc = _CACHE[key]

    in_maps = []
    for i in range(NCORES):
        xs = x[i * BLOC : (i + 1) * BLOC]  # [64, 512]
        m = {
            "v0": v0_dev,
            "v": v_dev,
            "h0": xs.astype(_BF16),
            "h0T": np.ascontiguousarray(
                xs.reshape(BLOC, KC, P).transpose(2, 1, 0).reshape(P, KC * BLOC)
            ).astype(_BF16),
        }
        if has_bias:
            m["bias"] = np.ascontiguousarray(
                np.broadcast_to(bias[None, :], (BLOC, GW))
            ).astype(np.float32)
        in_maps.append(m)

    res = run_bass_kernel_spmd(
        nc, in_maps, core_ids=list(range(NCORES)), trace=TRACE, tmpdir=TMPDIR
    )
    LAST["exec_time_ns"] = res.exec_time_ns
    LAST["results"] = res
    out = np.concatenate([res.results[i]["out"] for i in range(NCORES)], axis=0)
    return out.astype(np.float32)
